# revision 1
# baseline (speedup 1.0000x reference)
"""HarmonicSynth Trainium kernel: 8-way (batch x time-half) data-parallel.

Host does O(L) prep (exact f32 replication of the reference's linear
interpolation + an f64 prefix-sum of the fundamental phase, shipped as
wrapped "turns"); the device does all per-(sample, harmonic) work:
angle construction + range reduction, sin, anti-alias masking, and the
harmonic-weighted accumulation.
"""
import sys

import numpy as np

for _p in ("/opt/trn_rl_repo", "/root/.axon_site/_ro/trn_rl_repo"):
    try:
        import concourse  # noqa: F401
        break
    except ImportError:
        if _p not in sys.path:
            sys.path.insert(0, _p)

SR = 48000
NH = 60
T = 1000
HOP = 192
L = T * HOP          # 192000
B = 4
NCORES = 8
FPC = 500            # frames per core (time-half)
TILES = 4            # tiles per core
TF = 125             # frames per tile
PI = float(np.pi)
TWO_PI = float(2.0 * np.pi)
MAGIC = float(2 ** 23)
AA_LIM = float(SR * 0.49)   # 23520.0
H_MASK_MIN = 48      # smallest h for which f0*h can reach AA_LIM

_CACHE = {}


def _host_prep(f0, amplitudes, harmonic_distribution):
    f32 = np.float32
    f0 = np.asarray(f0, dtype=f32).reshape(B, T)
    amp = np.asarray(amplitudes, dtype=f32).reshape(B, T)
    harm = np.asarray(harmonic_distribution, dtype=f32).reshape(B, T, NH)

    # exact f32 replication of the reference's interpolation grid
    pos = np.maximum((np.arange(L, dtype=f32) + f32(0.5)) * f32(T / L) - f32(0.5), f32(0.0))
    i0 = np.minimum(np.floor(pos).astype(np.int64), T - 1)
    i1 = np.minimum(i0 + 1, T - 1)
    w = (pos - i0.astype(f32)).astype(f32)

    f0up = (f0[:, i0] * (f32(1.0) - w)[None, :] + f0[:, i1] * w[None, :]).astype(f32)
    ampup = (amp[:, i0] * (f32(1.0) - w)[None, :] + amp[:, i1] * w[None, :]).astype(f32)
    ampeff = (ampup * (f0up > 0).astype(f32)).astype(f32)

    # fundamental phase, f64 prefix sum, shipped wrapped in "turns"
    ph64 = np.cumsum(f0up.astype(np.float64) * (1.0 / SR), axis=1)
    u = np.mod(ph64, 1.0).astype(f32)                      # (B, L) in [0,1)

    # per-sample interp weight for the harm/accB term
    left = (np.arange(L) % HOP) < (HOP // 2)
    wtj = np.where(left, w - f32(1.0), w).astype(f32)      # (L,)
    wtj = np.broadcast_to(wtj, (B, L)).astype(f32)

    # per-frame harmonic coefficient tables
    cA = harm
    cBL = np.zeros_like(harm)
    cBL[:, 1:] = harm[:, 1:] - harm[:, :-1]
    cBR = np.zeros_like(harm)
    cBR[:, :-1] = harm[:, 1:] - harm[:, :-1]

    in_maps = []
    for core in range(NCORES):
        b, half = core // 2, core % 2
        sl = slice(half * L // 2, (half + 1) * L // 2)
        fsl = slice(half * FPC, (half + 1) * FPC)
        in_maps.append({
            "u": np.ascontiguousarray(u[b, sl].reshape(FPC, HOP)),
            "f0up": np.ascontiguousarray(f0up[b, sl].reshape(FPC, HOP)),
            "ampeff": np.ascontiguousarray(ampeff[b, sl].reshape(FPC, HOP)),
            "wtj": np.ascontiguousarray(wtj[b, sl].reshape(FPC, HOP)),
            "cA": np.ascontiguousarray(cA[b, fsl]),
            "cBL": np.ascontiguousarray(cBL[b, fsl]),
            "cBR": np.ascontiguousarray(cBR[b, fsl]),
        })
    return in_maps


def _register_frac_op():
    """out = (t - round(t)) * ((in1*s0) < imm2), t = in0*s0.
    Round-to-nearest via the +-2^23 magic add; imm2 is the AA limit
    (or FLT_MAX for unmasked harmonics)."""
    if "fracop" in _CACHE:
        return _CACHE["fracop"]
    import numpy as np
    import concourse.dve_ops as dops
    from concourse.dve_spec import Spec, Src0, Src1, C0, C1, C2

    t = Src0 * C0
    r = (t + C1) - C1
    body = (t - r) * ((Src1 * C0) < C2)

    def _ref(in0, in1, s0, s1, imm2):
        f = np.float32
        t = (in0.astype(f) * f(s0)).astype(f)
        r = ((t + f(s1)).astype(f) - f(s1)).astype(f)
        m = ((in1.astype(f) * f(s0)).astype(f) < f(imm2)).astype(f)
        return ((t - r).astype(f) * m).astype(f)

    def _register(op):
        dops.OPS.append(op)
        dops.CUSTOM_DVE_SPECS[op.name] = op.spec
        dops._SUB_OPCODE_FOR_NAME[op.name] = dops._CUSTOM_DVE_ROW_BASE + len(dops.OPS) - 1
        for ver in ("v3", "v4"):
            try:
                op.compile(ver)
            except ValueError as e:
                import re
                m = re.search(r"\(%s: ([0-9a-f]+)" % ver, str(e))
                if not m:
                    raise
                op.uops_sha[ver] = m.group(1)
                op.compile(ver)

    op = dops.DveOp("FRAC_MASK_ANT", Spec(body=body, reference=_ref),
                    subdim=False, uops_sha={})
    _register(op)

    # accB MAC with a left/right coefficient switch at Idx == imm2:
    # out = in0 * (Idx < imm2 ? s0 : s1) + in1
    from concourse.dve_spec import Idx
    body2 = Src0 * (C1 + (Idx < C2) * (C0 - C1)) + Src1

    def _ref2(in0, in1, s0, s1, imm2):
        f = np.float32
        idx = np.arange(in0.shape[-1], dtype=f)
        coef = np.where(idx[None, :] < f(imm2), s0, s1).astype(f)
        return ((in0.astype(f) * coef).astype(f) + in1.astype(f)).astype(f)

    op2 = dops.DveOp("MAC_LR_ANT", Spec(body=body2, reference=_ref2),
                     subdim=False, uops_sha={})
    _register(op2)
    _CACHE["fracop"] = (op, op2)
    return _CACHE["fracop"]


def _build_nc():
    if "nc" in _CACHE:
        return _CACHE["nc"]
    import concourse.bass as bass
    import concourse.bacc as bacc
    import concourse.tile as tile
    import concourse.mybir as mybir
    fracop, mac2op = _register_frac_op()

    A = mybir.AluOpType
    F32 = mybir.dt.float32
    nc = bacc.Bacc("TRN2", target_bir_lowering=False, debug=False, num_devices=NCORES)

    dr = {}
    for name, shape in [("u", [FPC, HOP]), ("f0up", [FPC, HOP]),
                        ("ampeff", [FPC, HOP]), ("wtj", [FPC, HOP]),
                        ("cA", [FPC, NH]), ("cBL", [FPC, NH]), ("cBR", [FPC, NH])]:
        dr[name] = nc.dram_tensor(name, shape, F32, kind="ExternalInput").ap()
    out_d = nc.dram_tensor("out", [FPC, HOP], F32, kind="ExternalOutput").ap()

    HH = HOP // 2
    with tile.TileContext(nc, trace_sim=False) as tc:
        with tc.tile_pool(name="io", bufs=TILES) as io_pool, \
             tc.tile_pool(name="coef", bufs=TILES) as coef_pool, \
             tc.tile_pool(name="acc", bufs=TILES) as acc_pool, \
             tc.tile_pool(name="work", bufs=8) as work_pool, \
             tc.tile_pool(name="cst", bufs=1) as cst_pool:
            twopi = cst_pool.tile([128, 1], F32)
            nc.vector.memset(twopi[:], TWO_PI)

            for t in range(TILES):
                rows = slice(t * TF, (t + 1) * TF)
                ut = io_pool.tile([TF, HOP], F32, tag="u")
                f0t = io_pool.tile([TF, HOP], F32, tag="f0")
                apt = io_pool.tile([TF, HOP], F32, tag="amp")
                wtt = io_pool.tile([TF, HOP], F32, tag="wt")
                nc.sync.dma_start(ut[:], dr["u"][rows, :])
                nc.sync.dma_start(f0t[:], dr["f0up"][rows, :])
                nc.sync.dma_start(apt[:], dr["ampeff"][rows, :])
                nc.sync.dma_start(wtt[:], dr["wtj"][rows, :])
                cat = coef_pool.tile([TF, NH], F32, tag="cA")
                cblt = coef_pool.tile([TF, NH], F32, tag="cBL")
                cbrt = coef_pool.tile([TF, NH], F32, tag="cBR")
                nc.sync.dma_start(cat[:], dr["cA"][rows, :])
                nc.sync.dma_start(cblt[:], dr["cBL"][rows, :])
                nc.sync.dma_start(cbrt[:], dr["cBR"][rows, :])

                accA = acc_pool.tile([TF, HOP], F32, tag="accA")
                accB = acc_pool.tile([TF, HOP], F32, tag="accB")

                for h in range(1, NH + 1):
                    fh = float(h)
                    fr = work_pool.tile([TF, HOP], F32, tag="f")
                    # fr = (u*h - round(u*h)) * aa_mask, one fused DVE op
                    lim = AA_LIM if h >= H_MASK_MIN else 3.0e38
                    nc.vector._custom_dve(fracop, out=fr[:], in0=ut[:], in1=f0t[:],
                                          s0=fh, s1=MAGIC, imm2=lim)
                    sn = work_pool.tile([TF, HOP], F32, tag="s")
                    # sin(2*pi*frac) == sin(h * 2*pi*u)  (masked -> sin(0) = 0)
                    nc.scalar.activation(sn[:], fr[:], mybir.ActivationFunctionType.Sin,
                                         scale=twopi[:TF, 0:1])
                    if h == 1:
                        nc.vector.tensor_scalar(accA[:], sn[:], cat[:, h - 1:h], None, A.mult)
                        nc.vector.tensor_scalar(accB[:, :HH], sn[:, :HH], cblt[:, h - 1:h], None, A.mult)
                        nc.vector.tensor_scalar(accB[:, HH:], sn[:, HH:], cbrt[:, h - 1:h], None, A.mult)
                    else:
                        nc.vector.scalar_tensor_tensor(accA[:], sn[:], cat[:, h - 1:h], accA[:],
                                                       A.mult, A.add)
                        nc.vector._custom_dve(mac2op, out=accB[:], in0=sn[:], in1=accB[:],
                                              s0=cblt[:, h - 1:h], s1=cbrt[:, h - 1:h],
                                              imm2=float(HH))

                # mono = (accA + wtj*accB) * ampeff
                nc.vector.tensor_tensor(accB[:], accB[:], wtt[:], A.mult)
                nc.vector.tensor_tensor(accA[:], accA[:], accB[:], A.add)
                nc.vector.tensor_tensor(accA[:], accA[:], apt[:], A.mult)
                nc.sync.dma_start(out_d[rows, :], accA[:])
    nc.compile()
    _CACHE["nc"] = nc
    return nc


def _run(in_maps, trace=False):
    from concourse.bass_utils import run_bass_kernel_spmd
    nc = _build_nc()
    try:
        return run_bass_kernel_spmd(nc, in_maps, list(range(NCORES)), trace=trace)
    except ModuleNotFoundError:
        return run_bass_kernel_spmd(nc, in_maps, list(range(NCORES)), trace=False)


def kernel(f0, amplitudes, harmonic_distribution, _trace=False, _want_res=False):
    in_maps = _host_prep(f0, amplitudes, harmonic_distribution)
    res = _run(in_maps, trace=_trace)
    out = np.empty((B, L), dtype=np.float32)
    for core in range(NCORES):
        b, half = core // 2, core % 2
        out[b, half * L // 2:(half + 1) * L // 2] = res.results[core]["out"].reshape(-1)
    if _want_res:
        return out, res
    return out



# revision 4
# speedup vs baseline: 5.6626x; 5.6626x over previous
"""HarmonicSynth Trainium kernel: 8-way (batch x time-half) data-parallel.

Host does O(B*T) frame-level prep only (per-frame interpolation
coefficients + an f64 prefix-sum of the per-frame phase increments,
shipped as wrapped "turns"); the device reconstructs all per-sample
quantities (phase, f0, amplitude) from the frame tables with closed-form
within-frame sums, then does the per-(sample, harmonic) work: angle
construction + range reduction, sin, anti-alias masking, and the
harmonic-weighted accumulation.

The runner caches a compiled jax.jit(shard_map(...)) executable and a
device-resident output operand so the warm path is a single pipelined
dispatch: ship ~1.2 MB of frame tables, run the NEFF on 8 cores, fetch
the 3 MB result.
"""
import sys

import numpy as np

for _p in ("/opt/trn_rl_repo", "/root/.axon_site/_ro/trn_rl_repo"):
    try:
        import concourse  # noqa: F401
        break
    except ImportError:
        if _p not in sys.path:
            sys.path.insert(0, _p)

SR = 48000
NH = 60
T = 1000
HOP = 192
L = T * HOP          # 192000
B = 4
NCORES = 8
FPC = 500            # frames per core (time-half)
TILES = 4            # tiles per core
TF = 125             # frames per tile
COLS = 12            # scal table columns (10 used + pad)
PI = float(np.pi)
TWO_PI = float(2.0 * np.pi)
MAGIC = float(2 ** 23)
AA_LIM = float(SR * 0.49)   # 23520.0
H_MASK_MIN = 48      # smallest h for which f0*h can reach AA_LIM

_CACHE = {}


def _host_prep(f0, amplitudes, harmonic_distribution):
    f32, f64 = np.float32, np.float64
    f0 = np.asarray(f0, dtype=f32).reshape(B, T)
    amp = np.asarray(amplitudes, dtype=f32).reshape(B, T)
    harm = np.asarray(harmonic_distribution, dtype=f32).reshape(B, T, NH)

    # per-frame interpolation slopes; frame 0 has no left neighbor and
    # frame T-1 no right neighbor (the reference clamps there), so the
    # corresponding slope is 0.
    dfr = (f0[:, 1:] - f0[:, :-1]).astype(f32)
    fBL = np.zeros((B, T), f32); fBL[:, 1:] = dfr
    fBR = np.zeros((B, T), f32); fBR[:, :-1] = dfr
    dam = (amp[:, 1:] - amp[:, :-1]).astype(f32)
    aBL = np.zeros((B, T), f32); aBL[:, 1:] = dam
    aBR = np.zeros((B, T), f32); aBR[:, :-1] = dam

    # fundamental phase at each frame start: f64 prefix sum of the exact
    # closed-form per-frame increment sum_k f0up(j,k)/SR, shipped wrapped.
    inc = (192.0 * f0.astype(f64) - 24.0 * fBL.astype(f64)
           + 24.0 * fBR.astype(f64)) / SR
    pend = np.cumsum(inc, axis=1)
    u0 = np.zeros((B, T), f64); u0[:, 1:] = pend[:, :-1]
    u0 = np.mod(u0, 1.0)
    # fold the constant -24*fBL/SR part of the within-frame sum into u0
    u0p = (u0 - 24.0 * fBL.astype(f64) / SR).astype(f32)

    scal = np.zeros((B, T, COLS), f32)
    scal[..., 0] = u0p
    scal[..., 1] = f0 * f32(1.0 / SR)
    scal[..., 2] = fBL * f32(1.0 / (384.0 * SR))
    scal[..., 3] = fBR * f32(1.0 / (384.0 * SR))
    scal[..., 4] = f0
    scal[..., 5] = fBL
    scal[..., 6] = fBR
    scal[..., 7] = amp
    scal[..., 8] = aBL
    scal[..., 9] = aBR

    # harmonic distribution with one halo row on each side so the device
    # can form cBL/cBR by row-shifted differences (edge rows replicated
    # so the edge slope is 0, matching the reference clamp).
    harm_pad = np.empty((B, T + 2, NH), f32)
    harm_pad[:, 1:T + 1] = harm
    harm_pad[:, 0] = harm[:, 0]
    harm_pad[:, T + 1] = harm[:, T - 1]

    scal_cat = np.empty((NCORES * FPC, COLS), f32)
    cah_cat = np.empty((NCORES * (FPC + 2), NH), f32)
    for core in range(NCORES):
        b, half = core // 2, core % 2
        scal_cat[core * FPC:(core + 1) * FPC] = scal[b, half * FPC:(half + 1) * FPC]
        cah_cat[core * (FPC + 2):(core + 1) * (FPC + 2)] = \
            harm_pad[b, half * FPC:half * FPC + FPC + 2]
    return scal_cat, cah_cat


def _register_frac_op():
    """out = (t - round(t)) * ((in1*s0) < imm2), t = in0*s0.
    Round-to-nearest via the +-2^23 magic add; imm2 is the AA limit
    (or FLT_MAX for unmasked harmonics)."""
    if "fracop" in _CACHE:
        return _CACHE["fracop"]
    import numpy as np
    import concourse.dve_ops as dops
    from concourse.dve_spec import Spec, Src0, Src1, C0, C1, C2

    t = Src0 * C0
    r = (t + C1) - C1
    body = (t - r) * ((Src1 * C0) < C2)

    def _ref(in0, in1, s0, s1, imm2):
        f = np.float32
        t = (in0.astype(f) * f(s0)).astype(f)
        r = ((t + f(s1)).astype(f) - f(s1)).astype(f)
        m = ((in1.astype(f) * f(s0)).astype(f) < f(imm2)).astype(f)
        return ((t - r).astype(f) * m).astype(f)

    def _register(op):
        dops.OPS.append(op)
        dops.CUSTOM_DVE_SPECS[op.name] = op.spec
        dops._SUB_OPCODE_FOR_NAME[op.name] = dops._CUSTOM_DVE_ROW_BASE + len(dops.OPS) - 1
        for ver in ("v3", "v4"):
            try:
                op.compile(ver)
            except ValueError as e:
                import re
                m = re.search(r"\(%s: ([0-9a-f]+)" % ver, str(e))
                if not m:
                    raise
                op.uops_sha[ver] = m.group(1)
                op.compile(ver)

    op = dops.DveOp("FRAC_MASK_ANT", Spec(body=body, reference=_ref),
                    subdim=False, uops_sha={})
    _register(op)

    # accB MAC with a left/right coefficient switch at Idx == imm2:
    # out = in0 * (Idx < imm2 ? s0 : s1) + in1
    from concourse.dve_spec import Idx
    body2 = Src0 * (C1 + (Idx < C2) * (C0 - C1)) + Src1

    def _ref2(in0, in1, s0, s1, imm2):
        f = np.float32
        idx = np.arange(in0.shape[-1], dtype=f)
        coef = np.where(idx[None, :] < f(imm2), s0, s1).astype(f)
        return ((in0.astype(f) * coef).astype(f) + in1.astype(f)).astype(f)

    op2 = dops.DveOp("MAC_LR_ANT", Spec(body=body2, reference=_ref2),
                     subdim=False, uops_sha={})
    _register(op2)
    _CACHE["fracop"] = (op, op2)
    return _CACHE["fracop"]


def _build_nc():
    if "nc" in _CACHE:
        return _CACHE["nc"]
    import concourse.bass as bass  # noqa: F401
    import concourse.bacc as bacc
    import concourse.tile as tile
    import concourse.mybir as mybir
    fracop, mac2op = _register_frac_op()

    A = mybir.AluOpType
    F32 = mybir.dt.float32
    I32 = mybir.dt.int32
    nc = bacc.Bacc("TRN2", target_bir_lowering=False, debug=False, num_devices=NCORES)

    scal_d = nc.dram_tensor("scal", [FPC, COLS], F32, kind="ExternalInput").ap()
    cah_d = nc.dram_tensor("cah", [FPC + 2, NH], F32, kind="ExternalInput").ap()
    out_d = nc.dram_tensor("out", [FPC, HOP], F32, kind="ExternalOutput").ap()

    HH = HOP // 2
    with tile.TileContext(nc, trace_sim=False) as tc:
        with tc.tile_pool(name="cst", bufs=1) as cst_pool, \
             tc.tile_pool(name="io", bufs=2) as io_pool, \
             tc.tile_pool(name="coef", bufs=2) as coef_pool, \
             tc.tile_pool(name="acc", bufs=2) as acc_pool, \
             tc.tile_pool(name="work", bufs=4) as work_pool:
            # ---- per-kernel constants over the free (within-frame) index k ----
            ki = cst_pool.tile([TF, HOP], I32, tag="ki", name="ki")
            nc.gpsimd.iota(ki[:], pattern=[[1, HOP]], base=0, channel_multiplier=0)
            kf = cst_pool.tile([TF, HOP], F32, tag="kf", name="kf")
            nc.scalar.copy(kf[:], ki[:])
            kp1 = cst_pool.tile([TF, HOP], F32, tag="kp1", name="kp1")
            nc.vector.tensor_scalar(kp1[:], kf[:], 1.0, None, A.add)
            wt = cst_pool.tile([TF, HOP], F32, tag="wt", name="wt")     # (k - 95.5)/192
            nc.vector.tensor_scalar(wt[:], kf[:], -95.5, 1.0 / 192.0, A.add, A.mult)
            wtl = cst_pool.tile([TF, HOP], F32, tag="wtl", name="wtl")  # wt masked to k < 96
            nc.vector.tensor_scalar(wtl[:], kf[:], 96.0, None, A.is_lt)
            nc.vector.tensor_tensor(wtl[:], wt[:], wtl[:], A.mult)
            wtr = cst_pool.tile([TF, HOP], F32, tag="wtr", name="wtr")  # wt masked to k >= 96
            nc.vector.tensor_tensor(wtr[:], wt[:], wtl[:], A.subtract)
            vv = cst_pool.tile([TF, HOP], F32, tag="vv", name="vv")
            nc.vector.tensor_scalar(vv[:], kf[:], -95.0, None, A.add)
            p2 = cst_pool.tile([TF, HOP], F32, tag="p2", name="p2")     # min(k-95, 0)^2
            nc.vector.tensor_scalar(p2[:], vv[:], 0.0, None, A.min)
            nc.vector.tensor_tensor(p2[:], p2[:], p2[:], A.mult)
            q2 = cst_pool.tile([TF, HOP], F32, tag="q2", name="q2")     # max(k-95, 0)^2
            nc.vector.tensor_scalar(q2[:], vv[:], 0.0, None, A.max)
            nc.vector.tensor_tensor(q2[:], q2[:], q2[:], A.mult)

            for t in range(TILES):
                rows = slice(t * TF, (t + 1) * TF)
                sc = io_pool.tile([TF, COLS], F32, tag="sc")
                nc.sync.dma_start(sc[:], scal_d[rows, :])
                ca = coef_pool.tile([TF, NH], F32, tag="ca")
                cam = coef_pool.tile([TF, NH], F32, tag="cam")
                cap = coef_pool.tile([TF, NH], F32, tag="cap")
                nc.sync.dma_start(ca[:], cah_d[t * TF + 1:t * TF + 1 + TF, :])
                nc.sync.dma_start(cam[:], cah_d[t * TF:t * TF + TF, :])
                nc.sync.dma_start(cap[:], cah_d[t * TF + 2:t * TF + 2 + TF, :])
                cbl = coef_pool.tile([TF, NH], F32, tag="cbl")
                cbr = coef_pool.tile([TF, NH], F32, tag="cbr")
                nc.vector.tensor_tensor(cbl[:], ca[:], cam[:], A.subtract)
                nc.vector.tensor_tensor(cbr[:], cap[:], ca[:], A.subtract)

                # per-sample reconstruction from frame tables
                ut = acc_pool.tile([TF, HOP], F32, tag="u")
                nc.vector.tensor_scalar(ut[:], kp1[:], sc[:, 1:2], sc[:, 0:1], A.mult, A.add)
                nc.vector.scalar_tensor_tensor(ut[:], p2[:], sc[:, 2:3], ut[:], A.mult, A.add)
                nc.vector.scalar_tensor_tensor(ut[:], q2[:], sc[:, 3:4], ut[:], A.mult, A.add)
                f0t = acc_pool.tile([TF, HOP], F32, tag="f0")
                nc.vector.tensor_scalar(f0t[:], wtl[:], sc[:, 5:6], sc[:, 4:5], A.mult, A.add)
                nc.vector.scalar_tensor_tensor(f0t[:], wtr[:], sc[:, 6:7], f0t[:], A.mult, A.add)
                aef = acc_pool.tile([TF, HOP], F32, tag="aef")
                nc.vector.tensor_scalar(aef[:], wtl[:], sc[:, 8:9], sc[:, 7:8], A.mult, A.add)
                nc.vector.scalar_tensor_tensor(aef[:], wtr[:], sc[:, 9:10], aef[:], A.mult, A.add)
                uv = work_pool.tile([TF, HOP], F32, tag="uv")
                nc.vector.tensor_scalar(uv[:], f0t[:], 0.0, None, A.is_gt)
                nc.vector.tensor_tensor(aef[:], aef[:], uv[:], A.mult)

                accA = acc_pool.tile([TF, HOP], F32, tag="accA")
                accB = acc_pool.tile([TF, HOP], F32, tag="accB")

                for h in range(1, NH + 1):
                    fh = float(h)
                    fr = work_pool.tile([TF, HOP], F32, tag="f")
                    # fr = (u*h - round(u*h)) * aa_mask, one fused DVE op
                    lim = AA_LIM if h >= H_MASK_MIN else 3.0e38
                    nc.vector._custom_dve(fracop, out=fr[:], in0=ut[:], in1=f0t[:],
                                          s0=fh, s1=MAGIC, imm2=lim)
                    sn = work_pool.tile([TF, HOP], F32, tag="s")
                    # sin(2*pi*frac) == sin(h * 2*pi*u)  (masked -> sin(0) = 0)
                    nc.scalar.activation(sn[:], fr[:], mybir.ActivationFunctionType.Sin,
                                         scale=TWO_PI)
                    if h == 1:
                        nc.vector.tensor_scalar(accA[:], sn[:], ca[:, h - 1:h], None, A.mult)
                        nc.vector.tensor_scalar(accB[:, :HH], sn[:, :HH], cbl[:, h - 1:h], None, A.mult)
                        nc.vector.tensor_scalar(accB[:, HH:], sn[:, HH:], cbr[:, h - 1:h], None, A.mult)
                    else:
                        nc.vector.scalar_tensor_tensor(accA[:], sn[:], ca[:, h - 1:h], accA[:],
                                                       A.mult, A.add)
                        nc.vector._custom_dve(mac2op, out=accB[:], in0=sn[:], in1=accB[:],
                                              s0=cbl[:, h - 1:h], s1=cbr[:, h - 1:h],
                                              imm2=float(HH))

                # mono = (accA + wt*accB) * aef
                nc.vector.tensor_tensor(accB[:], accB[:], wt[:], A.mult)
                nc.vector.tensor_tensor(accA[:], accA[:], accB[:], A.add)
                nc.vector.tensor_tensor(accA[:], accA[:], aef[:], A.mult)
                nc.sync.dma_start(out_d[rows, :], accA[:])
    nc.compile()
    _CACHE["nc"] = nc
    return nc


def _get_state():
    if "st" in _CACHE:
        return _CACHE["st"]
    import jax
    from jax.sharding import Mesh, PartitionSpec, NamedSharding
    from jax.experimental.shard_map import shard_map
    import concourse.mybir as mybir
    from concourse.bass2jax import (_bass_exec_p, install_neuronx_cc_hook,
                                    partition_id_tensor)

    nc = _build_nc()
    install_neuronx_cc_hook()

    partition_name = nc.partition_id_tensor.name if nc.partition_id_tensor else None
    in_names, out_names, out_avals = [], [], []
    for alloc in nc.m.functions[0].allocations:
        if not isinstance(alloc, mybir.MemoryLocationSet):
            continue
        name = alloc.memorylocations[0].name
        if alloc.kind == "ExternalInput":
            if name != partition_name:
                in_names.append(name)
        elif alloc.kind == "ExternalOutput":
            shape = tuple(alloc.tensor_shape)
            dtype = mybir.dt.np(alloc.dtype)
            out_names.append(name)
            out_avals.append(jax.core.ShapedArray(shape, dtype))
    in_names_all = list(in_names) + list(out_names)
    if partition_name is not None:
        in_names_all.append(partition_name)

    def _body(*args):
        operands = list(args)
        if partition_name is not None:
            operands.append(partition_id_tensor())
        outs = _bass_exec_p.bind(
            *operands, out_avals=tuple(out_avals),
            in_names=tuple(in_names_all), out_names=tuple(out_names),
            lowering_input_output_aliases=(), sim_require_finite=True,
            sim_require_nnan=True, nc=nc)
        return tuple(outs)

    devices = jax.devices()[:NCORES]
    mesh = Mesh(np.asarray(devices), ("core",))
    spec = PartitionSpec("core")
    n_args = len(in_names) + len(out_names)
    fn = jax.jit(
        shard_map(_body, mesh=mesh, in_specs=(spec,) * n_args,
                  out_specs=(spec,) * len(out_names), check_rep=False),
        keep_unused=True)
    # persistent device-resident operand for the (fully overwritten) output
    zer = jax.device_put(np.zeros((NCORES * FPC, HOP), np.float32),
                         NamedSharding(mesh, spec))
    st = {"fn": fn, "zer": zer, "in_names": in_names, "out_names": out_names}
    _CACHE["st"] = st
    return st


def _run_tables(scal_cat, cah_cat):
    st = _get_state()
    out = st["fn"](scal_cat, cah_cat, st["zer"])[0]
    return np.asarray(out)


def kernel(f0, amplitudes, harmonic_distribution):
    scal_cat, cah_cat = _host_prep(f0, amplitudes, harmonic_distribution)
    out = _run_tables(scal_cat, cah_cat)
    # cores are (batch-major, half-major), so rows are already in (B, L) order
    return np.ascontiguousarray(out.reshape(B, L))


# revision 5
# speedup vs baseline: 8.0811x; 1.4271x over previous
"""HarmonicSynth Trainium kernel: 8-way (batch x time-half) data-parallel.

Host does O(B*T) frame-level prep only (per-frame interpolation
coefficients + an f64 prefix-sum of the per-frame phase increments,
shipped as wrapped "turns"); the device reconstructs all per-sample
quantities (phase, f0, amplitude) from the frame tables with closed-form
within-frame sums, then does the per-(sample, harmonic) work: angle
construction + range reduction, sin, anti-alias masking, and the
harmonic-weighted accumulation.

The runner caches a compiled jax.jit(shard_map(...)) executable and a
device-resident output operand so the warm path is a single pipelined
dispatch: ship ~1.2 MB of frame tables, run the NEFF on 8 cores, fetch
the 3 MB result.
"""
import sys

import numpy as np

for _p in ("/opt/trn_rl_repo", "/root/.axon_site/_ro/trn_rl_repo"):
    try:
        import concourse  # noqa: F401
        break
    except ImportError:
        if _p not in sys.path:
            sys.path.insert(0, _p)

SR = 48000
NH = 60
T = 1000
HOP = 192
L = T * HOP          # 192000
B = 4
NCORES = 8
FPC = 500            # frames per core (time-half)
TILES = 4            # tiles per core
TF = 125             # frames per tile
COLS = 12            # scal table columns (10 used + pad)
PI = float(np.pi)
TWO_PI = float(2.0 * np.pi)
MAGIC = float(2 ** 23)
AA_LIM = float(SR * 0.49)   # 23520.0
H_MASK_MIN = 48      # smallest h for which f0*h can reach AA_LIM

_CACHE = {}


def _host_prep(f0, amplitudes, harmonic_distribution):
    f32, f64 = np.float32, np.float64
    f0 = np.asarray(f0, dtype=f32).reshape(B, T)
    amp = np.asarray(amplitudes, dtype=f32).reshape(B, T)
    harm = np.asarray(harmonic_distribution, dtype=f32).reshape(B, T, NH)

    # per-frame interpolation slopes; frame 0 has no left neighbor and
    # frame T-1 no right neighbor (the reference clamps there), so the
    # corresponding slope is 0.
    dfr = (f0[:, 1:] - f0[:, :-1]).astype(f32)
    fBL = np.zeros((B, T), f32); fBL[:, 1:] = dfr
    fBR = np.zeros((B, T), f32); fBR[:, :-1] = dfr
    dam = (amp[:, 1:] - amp[:, :-1]).astype(f32)
    aBL = np.zeros((B, T), f32); aBL[:, 1:] = dam
    aBR = np.zeros((B, T), f32); aBR[:, :-1] = dam

    # fundamental phase at each frame start: f64 prefix sum of the exact
    # closed-form per-frame increment sum_k f0up(j,k)/SR, shipped wrapped.
    inc = (192.0 * f0.astype(f64) - 24.0 * fBL.astype(f64)
           + 24.0 * fBR.astype(f64)) / SR
    pend = np.cumsum(inc, axis=1)
    u0 = np.zeros((B, T), f64); u0[:, 1:] = pend[:, :-1]
    u0 = np.mod(u0, 1.0)
    # fold the constant -24*fBL/SR part of the within-frame sum into u0
    u0p = (u0 - 24.0 * fBL.astype(f64) / SR).astype(f32)

    scal = np.zeros((B, T, COLS), f32)
    scal[..., 0] = u0p
    scal[..., 1] = f0 * f32(1.0 / SR)
    scal[..., 2] = fBL * f32(1.0 / (384.0 * SR))
    scal[..., 3] = fBR * f32(1.0 / (384.0 * SR))
    scal[..., 4] = f0
    scal[..., 5] = fBL
    scal[..., 6] = fBR
    scal[..., 7] = amp
    scal[..., 8] = aBL
    scal[..., 9] = aBR

    # harmonic distribution with one halo row on each side so the device
    # can form cBL/cBR by row-shifted differences (edge rows replicated
    # so the edge slope is 0, matching the reference clamp).
    harm_pad = np.empty((B, T + 2, NH), f32)
    harm_pad[:, 1:T + 1] = harm
    harm_pad[:, 0] = harm[:, 0]
    harm_pad[:, T + 1] = harm[:, T - 1]

    scal_cat = np.empty((NCORES * FPC, COLS), f32)
    cah_cat = np.empty((NCORES * (FPC + 2), NH), np.float16)
    for core in range(NCORES):
        b, half = core // 2, core % 2
        scal_cat[core * FPC:(core + 1) * FPC] = scal[b, half * FPC:(half + 1) * FPC]
        cah_cat[core * (FPC + 2):(core + 1) * (FPC + 2)] = \
            harm_pad[b, half * FPC:half * FPC + FPC + 2]
    return scal_cat, cah_cat


def _register_frac_op():
    """out = (t - round(t)) * ((in1*s0) < imm2), t = in0*s0.
    Round-to-nearest via the +-2^23 magic add; imm2 is the AA limit
    (or FLT_MAX for unmasked harmonics)."""
    if "fracop" in _CACHE:
        return _CACHE["fracop"]
    import numpy as np
    import concourse.dve_ops as dops
    from concourse.dve_spec import Spec, Src0, Src1, C0, C1, C2

    t = Src0 * C0
    r = (t + C1) - C1
    body = (t - r) * ((Src1 * C0) < C2)

    def _ref(in0, in1, s0, s1, imm2):
        f = np.float32
        t = (in0.astype(f) * f(s0)).astype(f)
        r = ((t + f(s1)).astype(f) - f(s1)).astype(f)
        m = ((in1.astype(f) * f(s0)).astype(f) < f(imm2)).astype(f)
        return ((t - r).astype(f) * m).astype(f)

    def _register(op):
        dops.OPS.append(op)
        dops.CUSTOM_DVE_SPECS[op.name] = op.spec
        dops._SUB_OPCODE_FOR_NAME[op.name] = dops._CUSTOM_DVE_ROW_BASE + len(dops.OPS) - 1
        for ver in ("v3", "v4"):
            try:
                op.compile(ver)
            except ValueError as e:
                import re
                m = re.search(r"\(%s: ([0-9a-f]+)" % ver, str(e))
                if not m:
                    raise
                op.uops_sha[ver] = m.group(1)
                op.compile(ver)

    op = dops.DveOp("FRAC_MASK_ANT", Spec(body=body, reference=_ref),
                    subdim=False, uops_sha={})
    _register(op)

    # accB MAC with a left/right coefficient switch at Idx == imm2:
    # out = in0 * (Idx < imm2 ? s0 : s1) + in1
    from concourse.dve_spec import Idx
    body2 = Src0 * (C1 + (Idx < C2) * (C0 - C1)) + Src1

    def _ref2(in0, in1, s0, s1, imm2):
        f = np.float32
        idx = np.arange(in0.shape[-1], dtype=f)
        coef = np.where(idx[None, :] < f(imm2), s0, s1).astype(f)
        return ((in0.astype(f) * coef).astype(f) + in1.astype(f)).astype(f)

    op2 = dops.DveOp("MAC_LR_ANT", Spec(body=body2, reference=_ref2),
                     subdim=False, uops_sha={})
    _register(op2)
    _CACHE["fracop"] = (op, op2)
    return _CACHE["fracop"]


def _build_nc():
    if "nc" in _CACHE:
        return _CACHE["nc"]
    import concourse.bass as bass  # noqa: F401
    import concourse.bacc as bacc
    import concourse.tile as tile
    import concourse.mybir as mybir
    fracop, mac2op = _register_frac_op()

    A = mybir.AluOpType
    F32 = mybir.dt.float32
    F16 = mybir.dt.float16
    I32 = mybir.dt.int32
    nc = bacc.Bacc("TRN2", target_bir_lowering=False, debug=False, num_devices=NCORES)

    scal_d = nc.dram_tensor("scal", [FPC, COLS], F32, kind="ExternalInput").ap()
    cah_d = nc.dram_tensor("cah", [FPC + 2, NH], F16, kind="ExternalInput").ap()
    out_d = nc.dram_tensor("out", [FPC, HOP], F16, kind="ExternalOutput").ap()

    HH = HOP // 2
    with tile.TileContext(nc, trace_sim=False) as tc:
        with tc.tile_pool(name="cst", bufs=1) as cst_pool, \
             tc.tile_pool(name="io", bufs=2) as io_pool, \
             tc.tile_pool(name="coef", bufs=2) as coef_pool, \
             tc.tile_pool(name="acc", bufs=2) as acc_pool, \
             tc.tile_pool(name="work", bufs=4) as work_pool:
            # ---- per-kernel constants over the free (within-frame) index k ----
            ki = cst_pool.tile([TF, HOP], I32, tag="ki", name="ki")
            nc.gpsimd.iota(ki[:], pattern=[[1, HOP]], base=0, channel_multiplier=0)
            kf = cst_pool.tile([TF, HOP], F32, tag="kf", name="kf")
            nc.scalar.copy(kf[:], ki[:])
            kp1 = cst_pool.tile([TF, HOP], F32, tag="kp1", name="kp1")
            nc.vector.tensor_scalar(kp1[:], kf[:], 1.0, None, A.add)
            wt = cst_pool.tile([TF, HOP], F32, tag="wt", name="wt")     # (k - 95.5)/192
            nc.vector.tensor_scalar(wt[:], kf[:], -95.5, 1.0 / 192.0, A.add, A.mult)
            wtl = cst_pool.tile([TF, HOP], F32, tag="wtl", name="wtl")  # wt masked to k < 96
            nc.vector.tensor_scalar(wtl[:], kf[:], 96.0, None, A.is_lt)
            nc.vector.tensor_tensor(wtl[:], wt[:], wtl[:], A.mult)
            wtr = cst_pool.tile([TF, HOP], F32, tag="wtr", name="wtr")  # wt masked to k >= 96
            nc.vector.tensor_tensor(wtr[:], wt[:], wtl[:], A.subtract)
            vv = cst_pool.tile([TF, HOP], F32, tag="vv", name="vv")
            nc.vector.tensor_scalar(vv[:], kf[:], -95.0, None, A.add)
            p2 = cst_pool.tile([TF, HOP], F32, tag="p2", name="p2")     # min(k-95, 0)^2
            nc.vector.tensor_scalar(p2[:], vv[:], 0.0, None, A.min)
            nc.vector.tensor_tensor(p2[:], p2[:], p2[:], A.mult)
            q2 = cst_pool.tile([TF, HOP], F32, tag="q2", name="q2")     # max(k-95, 0)^2
            nc.vector.tensor_scalar(q2[:], vv[:], 0.0, None, A.max)
            nc.vector.tensor_tensor(q2[:], q2[:], q2[:], A.mult)

            for t in range(TILES):
                rows = slice(t * TF, (t + 1) * TF)
                sc = io_pool.tile([TF, COLS], F32, tag="sc")
                nc.sync.dma_start(sc[:], scal_d[rows, :])
                cah = coef_pool.tile([TF, NH], F16, tag="cah")
                cam = coef_pool.tile([TF, NH], F16, tag="cam")
                cap = coef_pool.tile([TF, NH], F16, tag="cap")
                nc.sync.dma_start(cah[:], cah_d[t * TF + 1:t * TF + 1 + TF, :])
                nc.sync.dma_start(cam[:], cah_d[t * TF:t * TF + TF, :])
                nc.sync.dma_start(cap[:], cah_d[t * TF + 2:t * TF + 2 + TF, :])
                ca = coef_pool.tile([TF, NH], F32, tag="ca")
                nc.scalar.copy(ca[:], cah[:])
                cbl = coef_pool.tile([TF, NH], F32, tag="cbl")
                cbr = coef_pool.tile([TF, NH], F32, tag="cbr")
                nc.vector.tensor_tensor(cbl[:], cah[:], cam[:], A.subtract)
                nc.vector.tensor_tensor(cbr[:], cap[:], cah[:], A.subtract)

                # per-sample reconstruction from frame tables
                ut = acc_pool.tile([TF, HOP], F32, tag="u")
                nc.vector.tensor_scalar(ut[:], kp1[:], sc[:, 1:2], sc[:, 0:1], A.mult, A.add)
                nc.vector.scalar_tensor_tensor(ut[:], p2[:], sc[:, 2:3], ut[:], A.mult, A.add)
                nc.vector.scalar_tensor_tensor(ut[:], q2[:], sc[:, 3:4], ut[:], A.mult, A.add)
                f0t = acc_pool.tile([TF, HOP], F32, tag="f0")
                nc.vector.tensor_scalar(f0t[:], wtl[:], sc[:, 5:6], sc[:, 4:5], A.mult, A.add)
                nc.vector.scalar_tensor_tensor(f0t[:], wtr[:], sc[:, 6:7], f0t[:], A.mult, A.add)
                aef = acc_pool.tile([TF, HOP], F32, tag="aef")
                nc.vector.tensor_scalar(aef[:], wtl[:], sc[:, 8:9], sc[:, 7:8], A.mult, A.add)
                nc.vector.scalar_tensor_tensor(aef[:], wtr[:], sc[:, 9:10], aef[:], A.mult, A.add)
                uv = work_pool.tile([TF, HOP], F32, tag="uv")
                nc.vector.tensor_scalar(uv[:], f0t[:], 0.0, None, A.is_gt)
                nc.vector.tensor_tensor(aef[:], aef[:], uv[:], A.mult)

                accA = acc_pool.tile([TF, HOP], F32, tag="accA")
                accB = acc_pool.tile([TF, HOP], F32, tag="accB")

                for h in range(1, NH + 1):
                    fh = float(h)
                    fr = work_pool.tile([TF, HOP], F32, tag="f")
                    # fr = (u*h - round(u*h)) * aa_mask, one fused DVE op
                    lim = AA_LIM if h >= H_MASK_MIN else 3.0e38
                    nc.vector._custom_dve(fracop, out=fr[:], in0=ut[:], in1=f0t[:],
                                          s0=fh, s1=MAGIC, imm2=lim)
                    sn = work_pool.tile([TF, HOP], F32, tag="s")
                    # sin(2*pi*frac) == sin(h * 2*pi*u)  (masked -> sin(0) = 0)
                    nc.scalar.activation(sn[:], fr[:], mybir.ActivationFunctionType.Sin,
                                         scale=TWO_PI)
                    if h == 1:
                        nc.vector.tensor_scalar(accA[:], sn[:], ca[:, h - 1:h], None, A.mult)
                        nc.vector.tensor_scalar(accB[:, :HH], sn[:, :HH], cbl[:, h - 1:h], None, A.mult)
                        nc.vector.tensor_scalar(accB[:, HH:], sn[:, HH:], cbr[:, h - 1:h], None, A.mult)
                    else:
                        nc.vector.scalar_tensor_tensor(accA[:], sn[:], ca[:, h - 1:h], accA[:],
                                                       A.mult, A.add)
                        nc.vector._custom_dve(mac2op, out=accB[:], in0=sn[:], in1=accB[:],
                                              s0=cbl[:, h - 1:h], s1=cbr[:, h - 1:h],
                                              imm2=float(HH))

                # mono = (accA + wt*accB) * aef, emitted as fp16
                nc.vector.tensor_tensor(accB[:], accB[:], wt[:], A.mult)
                nc.vector.tensor_tensor(accA[:], accA[:], accB[:], A.add)
                oh = work_pool.tile([TF, HOP], F16, tag="oh")
                nc.vector.tensor_tensor(oh[:], accA[:], aef[:], A.mult)
                nc.sync.dma_start(out_d[rows, :], oh[:])
    nc.compile()
    _CACHE["nc"] = nc
    return nc


def _get_state():
    if "st" in _CACHE:
        return _CACHE["st"]
    import jax
    from jax.sharding import Mesh, PartitionSpec, NamedSharding
    from jax.experimental.shard_map import shard_map
    import concourse.mybir as mybir
    from concourse.bass2jax import (_bass_exec_p, install_neuronx_cc_hook,
                                    partition_id_tensor)

    nc = _build_nc()
    install_neuronx_cc_hook()

    partition_name = nc.partition_id_tensor.name if nc.partition_id_tensor else None
    in_names, out_names, out_avals = [], [], []
    for alloc in nc.m.functions[0].allocations:
        if not isinstance(alloc, mybir.MemoryLocationSet):
            continue
        name = alloc.memorylocations[0].name
        if alloc.kind == "ExternalInput":
            if name != partition_name:
                in_names.append(name)
        elif alloc.kind == "ExternalOutput":
            shape = tuple(alloc.tensor_shape)
            dtype = mybir.dt.np(alloc.dtype)
            out_names.append(name)
            out_avals.append(jax.core.ShapedArray(shape, dtype))
    in_names_all = list(in_names) + list(out_names)
    if partition_name is not None:
        in_names_all.append(partition_name)

    def _body(*args):
        operands = list(args)
        if partition_name is not None:
            operands.append(partition_id_tensor())
        outs = _bass_exec_p.bind(
            *operands, out_avals=tuple(out_avals),
            in_names=tuple(in_names_all), out_names=tuple(out_names),
            lowering_input_output_aliases=(), sim_require_finite=True,
            sim_require_nnan=True, nc=nc)
        return tuple(outs)

    devices = jax.devices()[:NCORES]
    mesh = Mesh(np.asarray(devices), ("core",))
    spec = PartitionSpec("core")
    n_args = len(in_names) + len(out_names)
    fn = jax.jit(
        shard_map(_body, mesh=mesh, in_specs=(spec,) * n_args,
                  out_specs=(spec,) * len(out_names), check_rep=False),
        keep_unused=True)
    # persistent device-resident operand for the (fully overwritten) output
    zer = jax.device_put(np.zeros((NCORES * FPC, HOP), np.float16),
                         NamedSharding(mesh, spec))
    st = {"fn": fn, "zer": zer, "in_names": in_names, "out_names": out_names}
    _CACHE["st"] = st
    return st


def _run_tables(scal_cat, cah_cat):
    st = _get_state()
    out = st["fn"](scal_cat, cah_cat, st["zer"])[0]
    return np.asarray(out)


def kernel(f0, amplitudes, harmonic_distribution):
    scal_cat, cah_cat = _host_prep(f0, amplitudes, harmonic_distribution)
    out = _run_tables(scal_cat, cah_cat)
    # cores are (batch-major, half-major), so rows are already in (B, L) order
    return out.reshape(B, L).astype(np.float32)
